# revision 24
# baseline (speedup 1.0000x reference)
"""FP8 semi-sparse (2:4) activation linear — Trainium2 Bass/Tile kernel.

Reference semantics:
  Wq, W_scale = rowwise fp8(e4m3fn) quant of weight      [N, K]
  Xq, X_scale = rowwise fp8(e4m3fn) quant of x           [M, K]
  Xsp         = 2:4 sparsify of Xq (keep 2 largest |.| per group of 4,
                ties -> earlier index)
  out         = (Xsp @ Wq^T) * X_scale * W_scale^T  -> bf16

Implementation (v2 — collective weight quantization):
  * Data-parallel over M (1024 x-rows/core) for the activation path; the
    WEIGHT path is sharded over cores: core c quantizes + transposes only W
    rows [512c, 512c+512), then an HBM AllGather shares the packed-fp8
    *transposed* weight (16 MB total) and row absmaxes with everyone.
    This removes 7/8 of the weight DVE/ScalarE/DMA work per core and all
    matmul-phase transposes (replaced by one 2 MB load per n-slice).
  * Halved-scale fp8 (TRN fp8e4 max 240 vs OCP 448): u = RNE(v/2), the x4
    folds into the output scales.
  * X path per 128-row tile: absmax (DVE) -> fp8 quant (ScalarE) -> 2:4
    selection on contiguous magnitude planes (DVE, >=2x modes) -> byte mask
    AND -> packed u16 to DRAM.  Per m-half: 16 xbar transposes
    [512,128]u16 -> [128,512] + ScalarE deinterleave into the persistent
    xspT2 [128,16,2,1024] fp8 k-parity planes.
  * Matmul: fp8 DoubleRow, contraction 256/instr; the rhs reads the raw
    interleaved pair layout via a strided [K,2,N] view (o-stride 1,
    n-stride 2) — no weight deinterleave.  16 k-pair matmuls accumulate
    into one PSUM bank per (n-slice, m-block) group.
  * Epilogue: one DVE scalar_tensor_tensor: out_bf16 = (psum*sx4[m])*swb.
  * Emission is software-pipelined: the own-slice W phase runs first (its
    gather overlaps the X phase), m-blocks 0-3 of every n-slice are matmul'd
    while X tiles 4-7 are still being selected, and the m 4-7 sweep streams
    after with the X-phase pools freed.
"""

import dataclasses

import numpy as np

import concourse.bass as bass
import concourse.mybir as mybir
import concourse.tile as tile
from concourse import bacc
from concourse.bass_utils import run_bass_kernel_spmd

P = 128
M_FULL, K_FULL, N_FULL = 8192, 4096, 4096
NCORES = 8
N_SLICE = 512

F32 = mybir.dt.float32
BF16 = mybir.dt.bfloat16
FP8 = mybir.dt.float8e4
U16 = mybir.dt.uint16

AX = mybir.AxisListType.X
OP = mybir.AluOpType
AF = mybir.ActivationFunctionType

SX_CONST = float(np.float32(4.0 / (448.0 * 448.0)))


def build_nc(m_core=M_FULL // NCORES, k=K_FULL, n=N_FULL) -> bass.Bass:
    assert m_core % P == 0 and k % (2 * P) == 0 and n % N_SLICE == 0
    m_tiles = m_core // P          # 8
    kp_tiles = k // (2 * P)        # 16
    n_slices = n // N_SLICE        # 8
    groups = k // 4                # 1024
    ku = k // 2                    # 2048 u16 per row
    mh0 = 5 * P                    # rows in first m-half (m-blocks 0-4)
    mh1 = m_core - mh0             # rows in second m-half (m-blocks 5-7)
    kw = kp_tiles * N_SLICE        # 8192 u16 per partition of one wt slice

    nc = bacc.Bacc(num_devices=NCORES)
    x = nc.declare_dram_parameter("x", [m_core, k], F32, isOutput=False)
    w = nc.declare_dram_parameter("weight", [n, k], F32, isOutput=False)
    out = nc.declare_dram_parameter("out", [m_core, n], BF16, isOutput=True)

    # collective buffers (HBM). inputs Local, outputs Shared.
    wq_own = nc.dram_tensor("wq_own", [N_SLICE, k], FP8)
    wtT_own = nc.dram_tensor("wtT_own", [P, kw], U16)
    wamax_own = nc.dram_tensor("wamax_own", [N_SLICE], F32)
    wtT_all = nc.dram_tensor("wtT_all", [n_slices, P, kw], U16, addr_space="Shared")
    wamax_all = nc.dram_tensor(
        "wamax_all", [n_slices, N_SLICE], F32, addr_space="Shared"
    )

    with tile.TileContext(nc) as tc:
        with (
            tc.tile_pool(name="dram", bufs=1, space="DRAM") as dpool,
            tc.tile_pool(name="small", bufs=8) as spool,
            tc.tile_pool(name="xld", bufs=3) as xldpool,
            tc.tile_pool(name="q8", bufs=2) as q8pool,
            tc.tile_pool(name="ob", bufs=3) as obpool,
            tc.tile_pool(name="persist", bufs=1) as perpool,
            tc.tile_pool(name="ps", bufs=1, space="PSUM") as pspool,
        ):
            xsp_dram = dpool.tile([m_core, k], FP8)
            xspT2 = perpool.tile([P, kp_tiles, 2, m_core], FP8)
            sx4 = perpool.tile([P, m_tiles], F32)
            swb_all = perpool.tile([P, n_slices, N_SLICE], F32)
            pss = [
                pspool.tile([P, N_SLICE], F32, tag=f"ps{m}", name=f"ps{m}")
                for m in range(m_tiles)
            ]
            pools = {}

            def quant_scale(t, tag, amax_out=None):
                amax = spool.tile([P, 1], F32, tag=f"am{tag}")
                nc.vector.tensor_reduce(
                    amax, t, axis=AX, op=OP.max, apply_absolute_value=True
                )
                if amax_out is None:
                    amax_out = spool.tile([P, 1], F32, tag=f"amc{tag}")
                nc.vector.tensor_scalar_max(amax_out, amax, 1e-12)
                rec = spool.tile([P, 1], F32, tag=f"rec{tag}")
                nc.vector.reciprocal(rec, amax_out)
                g = spool.tile([P, 1], F32, tag=f"g{tag}")
                nc.vector.tensor_scalar_mul(g, rec, 224.0)
                return g

            # -------- own-slice W phase: quant 512 rows, transpose, gather ----
            def load_w_own(j):
                t = xldpool.tile([P, k], F32, tag="xin")
                src = dataclasses.replace(w[0:P, :], offset=pid_off[0] + j * (P * k))
                nc.gpsimd.dma_start(t, src)
                return t

            def w_quant_own(j):
                t = w_loads.pop(j)
                amc = spool.tile([P, 1], F32, tag=f"amcw{j}")
                g = quant_scale(t, "w", amax_out=amc)
                wq8 = q8pool.tile([P, k], FP8, tag="q8")
                nc.scalar.activation(wq8, t, AF.Copy, scale=g)
                nc.sync.dma_start(wq_own[P * j : P * (j + 1), :], wq8)
                nc.sync.dma_start(wamax_own[P * j : P * (j + 1)], amc)

            def w_pack_and_gather():
                # transpose own 512 rows -> [128, 16, 512] u16, store packed
                wq_own_u16 = wq_own[:, :].bitcast(U16)  # [512, ku]
                wtT = pools["xt"].tile([P, kp_tiles, N_SLICE], U16, tag="xt")
                for t_ in range(kp_tiles):
                    nc.sync.dma_start_transpose(
                        wtT[:, t_, :], wq_own_u16[:, P * t_ : P * (t_ + 1)]
                    )
                nc.sync.dma_start(
                    wtT_own[:, :], wtT.rearrange("p t n -> p (t n)")
                )
                nc.gpsimd.collective_compute(
                    "AllGather",
                    mybir.AluOpType.bypass,
                    replica_groups=[list(range(NCORES))],
                    ins=[wtT_own[:, :].opt()],
                    outs=[wtT_all[:, :, :].opt()],
                )
                nc.gpsimd.collective_compute(
                    "AllGather",
                    mybir.AluOpType.bypass,
                    replica_groups=[list(range(NCORES))],
                    ins=[wamax_own[:].opt()],
                    outs=[wamax_all[:, :].opt()],
                )


            # ---------------- X tile: quantize + 2:4 select ----------------
            def load_x(mt):
                t = xldpool.tile([P, k], F32, tag="xin")
                nc.sync.dma_start(t, x[P * mt : P * (mt + 1), :])
                return t

            def x_tile(mt):
                t = x_loads.pop(mt)
                amax_c = spool.tile([P, 1], F32, tag="amcx")
                g = quant_scale(t, "x", amax_out=amax_c)
                nc.vector.tensor_scalar_mul(sx4[:, mt : mt + 1], amax_c, SX_CONST)
                xq = q8pool.tile([P, k], FP8, tag="q8")
                nc.scalar.activation(xq, t, AF.Copy, scale=g)
                xq16 = xq.bitcast(U16)
                xqv = xq16.rearrange("p (g t) -> p g t", t=2)

                e = pools["sel"].tile([P, 4, groups], U16, tag="e")
                nc.vector.tensor_scalar(
                    e[:, 0, :], xqv[:, :, 0], 0x007F, None, op0=OP.bitwise_and
                )
                nc.vector.tensor_scalar(
                    e[:, 1, :], xqv[:, :, 0], 8, 0x007F,
                    op0=OP.logical_shift_right, op1=OP.bitwise_and,
                )
                nc.vector.tensor_scalar(
                    e[:, 2, :], xqv[:, :, 1], 0x007F, None, op0=OP.bitwise_and
                )
                nc.vector.tensor_scalar(
                    e[:, 3, :], xqv[:, :, 1], 8, 0x007F,
                    op0=OP.logical_shift_right, op1=OP.bitwise_and,
                )

                b6 = pools["sel"].tile([P, 6, groups], U16, tag="b6")
                pairs = [(0, 1), (0, 2), (0, 3), (1, 2), (1, 3), (2, 3)]
                bidx = {}
                for pi, (i, j) in enumerate(pairs):
                    nc.vector.tensor_tensor(
                        b6[:, pi, :], e[:, i, :], e[:, j, :], op=OP.is_ge
                    )
                    bidx[(i, j)] = pi

                def b(i, j):
                    return b6[:, bidx[(i, j)], :]

                kk = pools["sel"].tile([P, 4, groups], BF16, tag="kk")
                s = pools["sel"].tile([P, 2, groups], BF16, tag="s")
                nc.vector.tensor_tensor(s[:, 0, :], b(0, 1), b(0, 2), op=OP.add)
                nc.vector.tensor_tensor(s[:, 0, :], s[:, 0, :], b(0, 3), op=OP.add)
                nc.vector.tensor_scalar(kk[:, 0, :], s[:, 0, :], 2.0, None, op0=OP.is_ge)
                nc.vector.tensor_tensor(s[:, 1, :], b(1, 2), b(1, 3), op=OP.add)
                nc.vector.tensor_tensor(s[:, 1, :], s[:, 1, :], b(0, 1), op=OP.subtract)
                nc.vector.tensor_scalar(kk[:, 1, :], s[:, 1, :], 1.0, None, op0=OP.is_ge)
                nc.vector.tensor_tensor(s[:, 0, :], b(2, 3), b(0, 2), op=OP.subtract)
                nc.vector.tensor_tensor(s[:, 0, :], s[:, 0, :], b(1, 2), op=OP.subtract)
                nc.vector.tensor_scalar(kk[:, 2, :], s[:, 0, :], 0.0, None, op0=OP.is_ge)
                nc.vector.tensor_tensor(s[:, 1, :], b(0, 3), b(1, 3), op=OP.add)
                nc.vector.tensor_tensor(s[:, 1, :], s[:, 1, :], b(2, 3), op=OP.add)
                nc.vector.tensor_scalar(kk[:, 3, :], s[:, 1, :], 1.0, None, op0=OP.is_le)

                nc.vector.tensor_scalar(
                    e[:, 0, :], kk[:, 1, :], 65280.0, None, op0=OP.mult
                )
                nc.vector.tensor_scalar(
                    e[:, 1, :], kk[:, 3, :], 65280.0, None, op0=OP.mult
                )
                mask = pools["sel"].tile([P, ku], U16, tag="mask")
                mv = mask.rearrange("p (g t) -> p g t", t=2)
                nc.vector.scalar_tensor_tensor(
                    mv[:, :, 0], kk[:, 0, :], 255.0, e[:, 0, :],
                    op0=OP.mult, op1=OP.add,
                )
                nc.vector.scalar_tensor_tensor(
                    mv[:, :, 1], kk[:, 2, :], 255.0, e[:, 1, :],
                    op0=OP.mult, op1=OP.add,
                )
                xsp = pools["xsp"].tile([P, ku], U16, tag="xsp")
                nc.vector.tensor_tensor(xsp, xq16, mask, op=OP.bitwise_and)
                nc.sync.dma_start(
                    xsp_dram.bitcast(U16)[P * mt : P * (mt + 1), :], xsp
                )

            # ---------------- X half: transpose + deinterleave ----------------
            xsp_u16 = xsp_dram.bitcast(U16)

            def x_half(h):
                r0, rows = (0, mh0) if h == 0 else (mh0, mh1)
                xt = pools["xt"].tile([P, kp_tiles, rows], U16, tag="xt", name=f"xt{h}")
                for t_ in range(kp_tiles):
                    nc.sync.dma_start_transpose(
                        xt[:, t_, :],
                        xsp_u16[r0 : r0 + rows, P * t_ : P * (t_ + 1)],
                    )
                xt8 = xt.bitcast(FP8).rearrange("p t (m o) -> p t m o", o=2)
                nc.scalar.activation(
                    xspT2[:, :, 0, r0 : r0 + rows], xt8[:, :, :, 0], AF.Copy
                )
                nc.scalar.activation(
                    xspT2[:, :, 1, r0 : r0 + rows], xt8[:, :, :, 1], AF.Copy
                )

            # ---------------- MM groups ----------------
            wt_slices = {}

            def load_wt(ns):
                wt = pools["wt"].tile([P, kp_tiles, N_SLICE], U16, tag="wt")
                nc.gpsimd.dma_start(
                    wt.rearrange("p t n -> p (t n)"), wtT_all[ns, :, :]
                )
                wt_slices[ns] = wt

            def mm_group(ns, m):
                wt = wt_slices[ns]
                ps = pss[m]
                for t_ in range(kp_tiles):
                    rhs = (
                        wt[:, t_, :]
                        .bitcast(FP8)
                        .rearrange("p (n o) -> p o n", o=2)
                    )
                    nc.tensor.matmul(
                        ps,
                        lhsT=xspT2[:, t_, :, P * m : P * (m + 1)],
                        rhs=rhs,
                        perf_mode=mybir.MatmulPerfMode.DoubleRow,
                        start=(t_ == 0),
                        stop=(t_ == kp_tiles - 1),
                    )
                ob = obpool.tile([P, N_SLICE], BF16, tag="ob")
                nc.vector.scalar_tensor_tensor(
                    ob, ps, sx4[:, m : m + 1], swb_all[:, ns, :],
                    op0=OP.mult, op1=OP.mult,
                )
                nc.gpsimd.dma_start(
                    out[P * m : P * (m + 1), N_SLICE * ns : N_SLICE * (ns + 1)], ob
                )

            # ---------------- emission ----------------
            x_loads, w_loads = {}, {}
            pid_off = [None]

            # slice pacing for the m0-3 sweep (backloaded: gather must land)
            SWEEP1 = {4: (0, 1), 5: (2, 3), 6: (4, 5), 7: (6, 7)}

            with (
                tc.tile_pool(name="sel", bufs=1) as _sel,
                tc.tile_pool(name="xsp", bufs=2) as _xsp,
                tc.tile_pool(name="xt", bufs=1) as _xt,
                tc.tile_pool(name="wtA", bufs=2) as _wtA,
            ):
                pools.update(sel=_sel, xsp=_xsp, xt=_xt, wt=_wtA)
                pid_off[0] = nc.partition_id() * (N_SLICE * k)
                x_loads[0] = load_x(0)
                w_loads[0] = load_w_own(0)
                w_loads[1] = load_w_own(1)
                w_quant_own(0)
                w_quant_own(1)
                w_loads[2] = load_w_own(2)
                w_loads[3] = load_w_own(3)
                w_quant_own(2)
                w_quant_own(3)
                w_pack_and_gather()
                x_loads[1] = load_x(1)
                x_tile(0)
                x_loads[2] = load_x(2)
                x_tile(1)
                x_loads[3] = load_x(3)
                x_tile(2)
                x_loads[4] = load_x(4)
                x_tile(3)
                # broadcast all slice amaxes into SBUF (needs gather #2)
                for ns_ in range(n_slices):
                    nc.scalar.dma_start(
                        swb_all[:, ns_, :],
                        wamax_all[ns_, :].unsqueeze(0).to_broadcast([P, N_SLICE]),
                    )
                for mt in range(4, m_tiles):
                    if mt + 1 < m_tiles:
                        x_loads[mt + 1] = load_x(mt + 1)
                    x_tile(mt)
                    if mt == 4:
                        x_half(0)
                    elif mt == 7:
                        x_half(1)
                    if mt >= 4:
                        for ns in SWEEP1[mt]:
                            load_wt(ns)
                            for m in range(5):
                                mm_group(ns, m)

            # post-X: m 4-7 sweep over all slices (wt reloaded, deeper pool)
            with tc.tile_pool(name="wtB", bufs=3) as _wtB:
                pools["wt"] = _wtB
                for ns in range(n_slices):
                    load_wt(ns)
                    for m in range(5, m_tiles):
                        mm_group(ns, m)

    return nc


_NC = None


def kernel(x: np.ndarray, weight: np.ndarray) -> np.ndarray:
    global _NC
    if _NC is None:
        _NC = build_nc()
        _NC.finalize()
    x = np.ascontiguousarray(x, dtype=np.float32)
    weight = np.ascontiguousarray(weight, dtype=np.float32)
    m_core = M_FULL // NCORES
    in_maps = [
        {"x": x[c * m_core : (c + 1) * m_core], "weight": weight}
        for c in range(NCORES)
    ]
    res = run_bass_kernel_spmd(_NC, in_maps, list(range(NCORES)))
    return np.concatenate([res.results[c]["out"] for c in range(NCORES)], axis=0)
